# revision 10
# baseline (speedup 1.0000x reference)
"""Distributed Trainium2 kernel for AttentionLayer+Experts.

Model: B=2, S=2048, D=1024, H=16 heads (DA=64), causal attention with
custom 1/(sqrt(64)*12) scale, residual gate, LayerNorm, then 4
sequence-chunk experts (FFN 1024->4096->1024, exact gelu), residual
with per-expert scalar, per-expert LayerNorm.

Sharding over 8 NeuronCores:
  - Attention: head-parallel. Core c computes heads {2c, 2c+1} for BOTH
    batches (perfect balance, no redundant compute).
  - One 8-rank AllToAll converts head-sharding -> sequence-sharding:
    core c ends up with (batch c//4, seq chunk c%4) which is exactly one
    expert's token chunk, so the expert FFN needs no further comm.
  - Compute in bf16 on TensorE with fp32 accumulation; LayerNorm math in
    fp32. Everything stays feature-major (transposed) so LN/bias terms
    are per-partition; final PE transpose produces the token-major
    output.
"""

import numpy as np
import ml_dtypes

BF16NP = ml_dtypes.bfloat16

B, S, D, H, DA, E = 2, 2048, 1024, 16, 64, 4
DFF = 4 * D
NCORES = 8
T = S // E  # 512 tokens per chunk / core
P = 128
SCALE = 1.0 / (np.sqrt(DA) * 12.0)
EPS = 1e-5
NDT = D // P      # 8 feature tiles
NQB = S // 512    # 4 query blocks per batch
NKT = S // P      # 16 key tiles per batch
NM1 = DFF // P    # 32 dff tiles
NTT = T // P      # 4 token tiles per chunk

_PROGRAM = None


def _build_program():
    from contextlib import ExitStack
    import concourse.bass as bass
    import concourse.mybir as mybir
    import concourse.tile as tile
    from concourse import bacc

    f32 = mybir.dt.float32
    bf = mybir.dt.bfloat16
    AF = mybir.ActivationFunctionType
    ALU = mybir.AluOpType

    nc = bacc.Bacc("TRN2", target_bir_lowering=False, debug=False,
                   num_devices=NCORES)

    def din(name, shape, dt):
        return nc.dram_tensor(name, shape, dt, kind="ExternalInput").ap()

    xT = din("xT", [B, NDT, P, S], bf)          # x transposed, both batches
    wq = din("wq", [NDT, P, P], bf)             # this core's 2 heads, [k,p,j]
    wk = din("wk", [NDT, P, P], bf)
    wv = din("wv", [NDT, P, P], bf)
    bqv = din("bq", [P, 1], f32)
    bkv = din("bk", [P, 1], f32)
    bvg = din("bvg", [P, 1], f32)               # gate * bv (2 heads)
    gate = din("gate", [1, 1], f32)             # residual gate scalar
    tri = din("tri", [P, P], bf)                # tri[p,f] = f>=p
    iden = din("iden", [P, P], f32)
    onesc_f = din("onesc_f", [P, 1], f32)
    onesc_b = din("onesc_b", [P, 1], bf)
    onesr_f = din("onesr_f", [1, P], f32)
    xcT = din("xcT", [NDT, P, T], f32)          # residual x^T for my chunk
    lng = din("lng", [P, NDT], f32)
    lnb = din("lnb", [P, NDT], f32)
    w1 = din("w1", [NDT, P, DFF], bf)           # my expert W1 [k,p,m]
    b1v = din("b1", [P, NM1], f32)
    w2 = din("w2", [NM1, P, D], bf)             # my expert W2 [k,p,m]
    b2s = din("b2s", [P, NDT], f32)             # e_scalar * b2
    esv = din("es", [P, 1], f32)                # e_scalar replicated
    elng = din("elng", [P, NDT], f32)
    elnb = din("elnb", [P, NDT], f32)
    out_d = nc.dram_tensor("out", [NTT, P, D], f32, kind="ExternalOutput").ap()

    with tile.TileContext(nc) as tc, ExitStack() as ctx:
        cpool = ctx.enter_context(tc.tile_pool(name="const", bufs=1))

        # ---- resident constants ----
        wq_sb = cpool.tile([P, NDT, P], bf)
        nc.sync.dma_start(wq_sb[:], wq.rearrange("k p j -> p k j"))
        wk_sb = cpool.tile([P, NDT, P], bf)
        nc.sync.dma_start(wk_sb[:], wk.rearrange("k p j -> p k j"))
        wv_sb = cpool.tile([P, NDT, P], bf)
        nc.sync.dma_start(wv_sb[:], wv.rearrange("k p j -> p k j"))
        bq_sb = cpool.tile([P, 1], f32)
        nc.sync.dma_start(bq_sb[:], bqv[:])
        bk_sb = cpool.tile([P, 1], f32)
        nc.sync.dma_start(bk_sb[:], bkv[:])
        bvg_sb = cpool.tile([P, 1], f32)
        nc.sync.dma_start(bvg_sb[:], bvg[:])
        gate_sb = cpool.tile([1, 1], f32)
        nc.sync.dma_start(gate_sb[:], gate[:])
        tri_sb = cpool.tile([P, P], bf)
        nc.sync.dma_start(tri_sb[:], tri[:])
        iden_sb = cpool.tile([P, P], f32)
        nc.sync.dma_start(iden_sb[:], iden[:])
        onescf_sb = cpool.tile([P, 1], f32)
        nc.sync.dma_start(onescf_sb[:], onesc_f[:])
        onescb_sb = cpool.tile([P, 1], bf)
        nc.sync.dma_start(onescb_sb[:], onesc_b[:])
        onesrf_sb = cpool.tile([1, P], f32)
        nc.sync.dma_start(onesrf_sb[:], onesr_f[:])
        lng_sb = cpool.tile([P, NDT], f32)
        nc.sync.dma_start(lng_sb[:], lng[:])
        lnb_sb = cpool.tile([P, NDT], f32)
        nc.sync.dma_start(lnb_sb[:], lnb[:])
        b1_sb = cpool.tile([P, NM1], f32)
        nc.sync.dma_start(b1_sb[:], b1v[:])
        b2s_sb = cpool.tile([P, NDT], f32)
        nc.sync.dma_start(b2s_sb[:], b2s[:])
        es_sb = cpool.tile([P, 1], f32)
        nc.sync.dma_start(es_sb[:], esv[:])
        elng_sb = cpool.tile([P, NDT], f32)
        nc.sync.dma_start(elng_sb[:], elng[:])
        elnb_sb = cpool.tile([P, NDT], f32)
        nc.sync.dma_start(elnb_sb[:], elnb[:])
        eps_sb = cpool.tile([1, 1], f32)
        nc.vector.memset(eps_sb[:], float(EPS))
        xc_sb = []
        for dt in range(NDT):
            t = cpool.tile([P, T], f32, tag="xc", bufs=NDT, name=f"xc{dt}")
            nc.sync.dma_start(t[:], xcT[dt])
            xc_sb.append(t)

        # a2a DRAM bounce buffers
        dpool = ctx.enter_context(
            tc.tile_pool(name="dramp", bufs=1, space="DRAM"))
        a_in = dpool.tile([NCORES, P, 512], bf)
        a_out = dpool.tile([NCORES, P, 512], bf)

        # =========== phase 1: projections + attention (per batch) ==========
        with tc.tile_pool(name="psA", bufs=1, space=bass.MemorySpace.PSUM) \
                as psA, \
             tc.tile_pool(name="xtp", bufs=NDT) as xtp, \
             tc.tile_pool(name="qkp", bufs=2) as qkp, \
             tc.tile_pool(name="vp", bufs=NKT) as vp, \
             tc.tile_pool(name="ep", bufs=4) as epool, \
             tc.tile_pool(name="smallp", bufs=12) as smp, \
             tc.tile_pool(name="stgp", bufs=3) as stgp:
            for b in range(B):
                xt_b = []
                for dt in range(NDT):
                    t = xtp.tile([P, S], bf, tag="xt", name=f"xt{b}_{dt}")
                    nc.sync.dma_start(t[:], xT[b, dt])
                    xt_b.append(t)

                # q^T, k^T: [128(2h x 64), S]
                qT = qkp.tile([P, S], bf, tag="qT", name=f"qT{b}")
                kT = qkp.tile([P, S], bf, tag="kT", name=f"kT{b}")
                for (w_sb, b_sb, oT) in ((wq_sb, bq_sb, qT),
                                         (wk_sb, bk_sb, kT)):
                    for qb in range(NQB):
                        q0 = 512 * qb
                        ps = psA.tile([P, 512], f32, tag="proj", bufs=2,
                                      name=f"pj{b}{qb}")
                        for k in range(NDT):
                            nc.tensor.matmul(
                                ps[:], w_sb[:, k, :], xt_b[k][:, q0:q0 + 512],
                                start=(k == 0), stop=(k == NDT - 1))
                        nc.vector.tensor_scalar_add(
                            oT[:, q0:q0 + 512], ps[:], b_sb[:])

                # v (token-major) with ones column per head: [128, 2*65]
                v_b = []
                for tt in range(NKT):
                    t0 = P * tt
                    ps = psA.tile([P, P], f32, tag="proj", bufs=2,
                                  name=f"pv{b}{tt}")
                    for k in range(NDT):
                        nc.tensor.matmul(
                            ps[:], xt_b[k][:, t0:t0 + P], wv_sb[:, k, :],
                            start=(k == 0), stop=(k == NDT - 1))
                    vt = vp.tile([P, 130], bf, tag="v", name=f"v{b}_{tt}")
                    nc.vector.memset(vt[:], 1.0)
                    nc.vector.tensor_copy(vt[:, 0:64], ps[:, 0:64])
                    nc.vector.tensor_copy(vt[:, 65:129], ps[:, 64:128])
                    v_b.append(vt)

                # attention per (qb, h); destination core d = b*4 + qb
                for qb in range(NQB):
                    q0 = 512 * qb
                    nkt = 4 * (qb + 1)
                    stg = stgp.tile([P, 512], bf, tag="stg",
                                    name=f"stg{b}{qb}")
                    for h in range(2):
                        hp = h * 64
                        o_ps = psA.tile([65, 512], f32, tag="o", bufs=2,
                                        name=f"o{b}{qb}{h}")
                        for kt in range(nkt):
                            k0 = P * kt
                            off = max(0, k0 - q0)
                            n = 512 - off
                            s_ps = psA.tile([P, 512], f32, tag="sc", bufs=2,
                                            name=f"s{b}{qb}{h}{kt}")
                            nc.tensor.matmul(
                                s_ps[:, 0:n],
                                kT[hp:hp + 64, k0:k0 + P],
                                qT[hp:hp + 64, q0 + off:q0 + 512],
                                start=True, stop=True)
                            e_sb = epool.tile([P, 512], bf, tag="exp",
                                              name=f"e{b}{qb}{h}{kt}")
                            nc.scalar.activation(
                                e_sb[:, 0:n], s_ps[:, 0:n], AF.Exp,
                                bias=0.0, scale=float(SCALE))
                            if k0 >= q0:  # diagonal block: causal mask
                                nc.vector.tensor_mul(
                                    e_sb[:, 0:P], e_sb[:, 0:P], tri_sb[:])
                            nc.tensor.matmul(
                                o_ps[:, off:512],
                                v_b[kt][:, h * 65:h * 65 + 65],
                                e_sb[:, 0:n],
                                start=(kt == 0), stop=(kt == nkt - 1))
                        # normalize: row 64 of o_ps is the softmax denom
                        recip = smp.tile([1, 512], f32, tag="sm",
                                         name=f"rc{b}{qb}{h}")
                        nc.vector.reciprocal(recip[:], o_ps[64:65, :])
                        rg = smp.tile([1, 512], f32, tag="sm",
                                      name=f"rg{b}{qb}{h}")
                        nc.vector.tensor_scalar_mul(rg[:], recip[:],
                                                    gate_sb[:])
                        rep = psA.tile([64, 512], f32, tag="rep", bufs=2,
                                       name=f"rp{b}{qb}{h}")
                        nc.tensor.matmul(rep[:], onesrf_sb[:, 0:64], rg[:],
                                         start=True, stop=True)
                        rep_sb = epool.tile([64, 512], f32, tag="repsb",
                                            bufs=2, name=f"rs{b}{qb}{h}")
                        nc.vector.tensor_copy(rep_sb[:], rep[:])
                        nc.vector.tensor_mul(stg[hp:hp + 64, :],
                                             o_ps[0:64, :], rep_sb[:])
                        nc.vector.tensor_scalar_add(
                            stg[hp:hp + 64, :], stg[hp:hp + 64, :],
                            bvg_sb[hp:hp + 64, :])
                    nc.sync.dma_start(a_in[b * NQB + qb], stg[:])

        # =========== phase 2: AllToAll ==========
        nc.gpsimd.collective_compute(
            "AllToAll", mybir.AluOpType.bypass,
            replica_groups=[list(range(NCORES))],
            ins=[a_in[:].opt()], outs=[a_out[:].opt()])

        # =========== phase 3: residual + LN1 (feature-major) ==========
        x1f = []   # fp32, becomes x1 after LN
        x1b = []   # bf16 copy for FFN rhs
        lnp = ctx.enter_context(tc.tile_pool(name="lnp", bufs=1))
        aop = ctx.enter_context(tc.tile_pool(name="aop", bufs=4))
        for dt in range(NDT):
            ao = aop.tile([P, 512], bf, tag="ao", name=f"ao{dt}")
            nc.sync.dma_start(ao[:], a_out[dt])
            xf = lnp.tile([P, T], f32, tag="x1f", bufs=NDT, name=f"x1f{dt}")
            nc.vector.tensor_add(xf[:], xc_sb[dt][:], ao[:])
            x1f.append(xf)
            x1b.append(lnp.tile([P, T], bf, tag="x1b", bufs=NDT,
                                name=f"x1b{dt}"))

        def ln_T(x_tiles, g_sb, b_sb, psum_pool, bf_out, nm):
            mean_ps = psum_pool.tile([1, 512], f32, tag="red", bufs=2,
                                     name=f"mn{nm}")
            for dt in range(NDT):
                nc.tensor.matmul(mean_ps[:], onescf_sb[:], x_tiles[dt][:],
                                 start=(dt == 0), stop=(dt == NDT - 1))
            sq_ps = psum_pool.tile([1, 512], f32, tag="red", bufs=2,
                                   name=f"sq{nm}")
            for dt in range(NDT):
                sq = smp2.tile([P, T], bf, tag="sqt", bufs=3,
                               name=f"sqt{nm}{dt}")
                nc.vector.tensor_mul(sq[:], x_tiles[dt][:], x_tiles[dt][:])
                nc.tensor.matmul(sq_ps[:], onescb_sb[:], sq[:],
                                 start=(dt == 0), stop=(dt == NDT - 1))
            mu = smp2.tile([1, 512], f32, tag="sm2", bufs=8, name=f"mu{nm}")
            nc.vector.tensor_scalar_mul(mu[:], mean_ps[:], 1.0 / D)
            ex2 = smp2.tile([1, 512], f32, tag="sm2", bufs=8, name=f"e2{nm}")
            nc.vector.tensor_scalar_mul(ex2[:], sq_ps[:], 1.0 / D)
            mu2 = smp2.tile([1, 512], f32, tag="sm2", bufs=8, name=f"m2{nm}")
            nc.vector.tensor_mul(mu2[:], mu[:], mu[:])
            var = smp2.tile([1, 512], f32, tag="sm2", bufs=8, name=f"vr{nm}")
            nc.vector.tensor_sub(var[:], ex2[:], mu2[:])
            sig = smp2.tile([1, 512], f32, tag="sm2", bufs=8, name=f"sg{nm}")
            nc.scalar.activation(sig[:], var[:], AF.Sqrt, bias=eps_sb[:])
            rsig = smp2.tile([1, 512], f32, tag="sm2", bufs=8, name=f"rs{nm}")
            nc.vector.reciprocal(rsig[:], sig[:])
            mu_rep = psum_pool.tile([P, 512], f32, tag="rep2", bufs=2,
                                    name=f"mr{nm}")
            nc.tensor.matmul(mu_rep[:], onesrf_sb[:], mu[:],
                             start=True, stop=True)
            rs_rep = psum_pool.tile([P, 512], f32, tag="rep2", bufs=2,
                                    name=f"rr{nm}")
            nc.tensor.matmul(rs_rep[:], onesrf_sb[:], rsig[:],
                             start=True, stop=True)
            for dt in range(NDT):
                x = x_tiles[dt]
                nc.vector.tensor_sub(x[:], x[:], mu_rep[:])
                nc.vector.tensor_mul(x[:], x[:], rs_rep[:])
                nc.vector.tensor_scalar(
                    x[:], x[:], g_sb[:, dt:dt + 1], b_sb[:, dt:dt + 1],
                    ALU.mult, ALU.add)
                if bf_out is not None:
                    nc.vector.tensor_copy(bf_out[dt][:], x[:])

        smp2 = ctx.enter_context(tc.tile_pool(name="smp2", bufs=1))
        with tc.tile_pool(name="psB", bufs=1,
                          space=bass.MemorySpace.PSUM) as psB:
            ln_T(x1f, lng_sb, lnb_sb, psB, x1b, "a")

        # =========== phase 4: expert FFN ==========
        hp_pool = ctx.enter_context(tc.tile_pool(name="hT", bufs=NM1))
        hT = []
        zp = ctx.enter_context(tc.tile_pool(name="zp", bufs=NDT))
        wp = ctx.enter_context(tc.tile_pool(name="wp", bufs=1))
        # FFN1: groups of 4 dff-tiles; stream W1 slices
        with tc.tile_pool(name="psC", bufs=1,
                          space=bass.MemorySpace.PSUM) as psC:
            for mg in range(NM1 // 4):
                w1t = wp.tile([P, NDT, 512], bf, tag="w1", bufs=2,
                              name=f"w1t{mg}")
                nc.sync.dma_start(
                    w1t[:],
                    w1[:, :, mg * 512:(mg + 1) * 512]
                    .rearrange("k p j -> p k j"))
                fps = [psC.tile([P, T], f32, tag="f1", bufs=6,
                                name=f"f1_{mg}_{i}") for i in range(4)]
                for k in range(NDT):
                    for i in range(4):
                        nc.tensor.matmul(
                            fps[i][:], w1t[:, k, i * P:(i + 1) * P],
                            x1b[k][:],
                            start=(k == 0), stop=(k == NDT - 1))
                for i in range(4):
                    m = mg * 4 + i
                    ht = hp_pool.tile([P, T], bf, tag="hT", name=f"hT{m}")
                    nc.scalar.activation(ht[:], fps[i][:], AF.Gelu,
                                         bias=b1_sb[:, m:m + 1], scale=1.0)
                    hT.append(ht)

        # FFN2: all 8 output d-tiles accumulate across one W2 k-sweep
        z = []
        with tc.tile_pool(name="psD", bufs=1,
                          space=bass.MemorySpace.PSUM) as psD:
            yps = [psD.tile([P, T], f32, tag="f2", bufs=NDT, name=f"y{dt}")
                   for dt in range(NDT)]
            for k in range(NM1):
                w2t = wp.tile([P, D], bf, tag="w2", bufs=3, name=f"w2t{k}")
                nc.sync.dma_start(w2t[:], w2[k])
                for dt in range(NDT):
                    nc.tensor.matmul(
                        yps[dt][:], w2t[:, dt * P:(dt + 1) * P], hT[k][:],
                        start=(k == 0), stop=(k == NM1 - 1))
            for dt in range(NDT):
                zt = zp.tile([P, T], f32, tag="z", bufs=NDT, name=f"z{dt}")
                # z = es*y + x1
                nc.vector.scalar_tensor_tensor(
                    zt[:], yps[dt][:], es_sb[:], x1f[dt][:],
                    ALU.mult, ALU.add)
                nc.vector.tensor_scalar_add(zt[:], zt[:],
                                            b2s_sb[:, dt:dt + 1])
                z.append(zt)

        # =========== phase 5: LN2 + transpose + output ==========
        with tc.tile_pool(name="psE", bufs=1,
                          space=bass.MemorySpace.PSUM) as psE:
            ln_T(z, elng_sb, elnb_sb, psE, None, "b")

        with tc.tile_pool(name="psF", bufs=1,
                          space=bass.MemorySpace.PSUM) as psF, \
             tc.tile_pool(name="outp", bufs=NTT) as outp:
            for tt in range(NTT):
                ot = outp.tile([P, D], f32, tag="ot", name=f"ot{tt}")
                for dt in range(NDT):
                    tp = psF.tile([P, P], f32, tag="tr", bufs=4,
                                  name=f"tr{tt}{dt}")
                    nc.tensor.transpose(tp[:], z[dt][:, tt * P:(tt + 1) * P],
                                        iden_sb[:])
                    nc.vector.tensor_copy(ot[:, dt * P:(dt + 1) * P], tp[:])
                nc.sync.dma_start(out_d[tt], ot[:])

    nc.compile()
    return nc


def _get_program():
    global _PROGRAM
    if _PROGRAM is None:
        _PROGRAM = _build_program()
    return _PROGRAM


def _host_prep(inputs):
    """Shard + lay out inputs for each of the 8 cores."""
    x = np.asarray(inputs["x"], np.float32)
    Wq = np.asarray(inputs["Wq"], np.float32)
    bq = np.asarray(inputs["bq"], np.float32)
    Wk = np.asarray(inputs["Wk"], np.float32)
    bk = np.asarray(inputs["bk"], np.float32)
    Wv = np.asarray(inputs["Wv"], np.float32)
    bv = np.asarray(inputs["bv"], np.float32)
    scalar = np.float32(inputs["scalar"])
    ln_g = np.asarray(inputs["ln_g"], np.float32)
    ln_b = np.asarray(inputs["ln_b"], np.float32)
    eW1 = np.asarray(inputs["eW1"], np.float32)
    eb1 = np.asarray(inputs["eb1"], np.float32)
    eW2 = np.asarray(inputs["eW2"], np.float32)
    eb2 = np.asarray(inputs["eb2"], np.float32)
    e_scalar = np.asarray(inputs["e_scalar"], np.float32)
    eln_g = np.asarray(inputs["eln_g"], np.float32)
    eln_b = np.asarray(inputs["eln_b"], np.float32)

    xT_all = np.ascontiguousarray(x.transpose(0, 2, 1)).reshape(B, NDT, P, S)
    xT_bf = xT_all.astype(BF16NP)
    tri = (np.arange(P)[None, :] >= np.arange(P)[:, None])
    iden = np.eye(P, dtype=np.float32)

    def col(v):
        return np.ascontiguousarray(v.reshape(-1, 1), dtype=np.float32)

    def pk(v):  # [D]-like -> [P, n]
        n = v.size // P
        return np.ascontiguousarray(v.reshape(n, P).T, dtype=np.float32)

    in_maps = []
    for c in range(NCORES):
        h0 = 2 * c
        b_out, e_out = c // NQB, c % NQB
        t0 = e_out * T
        wq_c = np.concatenate([Wq[h0], Wq[h0 + 1]], axis=1)  # [1024,128]
        wk_c = np.concatenate([Wk[h0], Wk[h0 + 1]], axis=1)
        wv_c = np.concatenate([Wv[h0], Wv[h0 + 1]], axis=1)
        bq_c = np.concatenate([bq[h0], bq[h0 + 1]])
        bk_c = np.concatenate([bk[h0], bk[h0 + 1]])
        bv_c = np.concatenate([bv[h0], bv[h0 + 1]])
        xc = np.ascontiguousarray(x[b_out, t0:t0 + T, :].T)  # [1024, 512]
        m = {
            "xT": xT_bf,
            "wq": np.ascontiguousarray(wq_c.reshape(NDT, P, P), BF16NP),
            "wk": np.ascontiguousarray(wk_c.reshape(NDT, P, P), BF16NP),
            "wv": np.ascontiguousarray(wv_c.reshape(NDT, P, P), BF16NP),
            "bq": col(bq_c),
            "bk": col(bk_c),
            "bvg": col(scalar * bv_c),
            "gate": np.full((1, 1), scalar, np.float32),
            "tri": tri.astype(BF16NP),
            "iden": iden,
            "onesc_f": np.ones((P, 1), np.float32),
            "onesc_b": np.ones((P, 1), BF16NP),
            "onesr_f": np.ones((1, P), np.float32),
            "xcT": np.ascontiguousarray(xc.reshape(NDT, P, T), np.float32),
            "lng": pk(ln_g),
            "lnb": pk(ln_b),
            "w1": np.ascontiguousarray(
                eW1[e_out].reshape(NDT, P, DFF), BF16NP),
            "b1": pk(eb1[e_out]),
            "w2": np.ascontiguousarray(
                eW2[e_out].reshape(NM1, P, D), BF16NP),
            "b2s": pk(e_scalar[e_out] * eb2[e_out]),
            "es": np.full((P, 1), e_scalar[e_out], np.float32),
            "elng": pk(eln_g[e_out]),
            "elnb": pk(eln_b[e_out]),
        }
        in_maps.append(m)
    return in_maps


_LAST_RESULT = {}


def kernel(**inputs) -> np.ndarray:
    import os
    from concourse.bass_utils import run_bass_kernel_spmd

    nc = _get_program()
    in_maps = _host_prep(inputs)
    trace = bool(int(os.environ.get("KBENCH_TRACE", "0")))
    res = run_bass_kernel_spmd(nc, in_maps, core_ids=list(range(NCORES)),
                               trace=trace)
    _LAST_RESULT["exec_time_ns"] = res.exec_time_ns
    _LAST_RESULT["res"] = res

    out = np.empty((B, S, D), np.float32)
    for c in range(NCORES):
        b_out, e_out = c // NQB, c % NQB
        chunk = np.asarray(res.results[c]["out"], np.float32)
        out[b_out, e_out * T:(e_out + 1) * T, :] = chunk.reshape(T, D)
    return out


# revision 12
# speedup vs baseline: 1.3278x; 1.3278x over previous
"""Distributed Trainium2 kernel for AttentionLayer+Experts.

Model: B=2, S=2048, D=1024, H=16 heads (DA=64), causal attention with
custom 1/(sqrt(64)*12) scale, residual gate, LayerNorm, then 4
sequence-chunk experts (FFN 1024->4096->1024, exact gelu), residual
with per-expert scalar, per-expert LayerNorm.

Sharding over 8 NeuronCores:
  - Attention: head-parallel. Core c computes heads {2c, 2c+1} for BOTH
    batches (perfect balance, no redundant compute).
  - One 8-rank AllToAll converts head-sharding -> sequence-sharding:
    core c ends up with (batch c//4, seq chunk c%4) which is exactly one
    expert's token chunk, so the expert FFN needs no further comm.
  - Compute in bf16 on TensorE with fp32 accumulation; LayerNorm math in
    fp32. Everything stays feature-major (transposed) so LN/bias terms
    are per-partition; final PE transpose produces the token-major
    output.
  - Softmax denominators ride along in the AV matmul via 64 ones
    columns appended to V (replicated rowsum rows for free), so the
    per-token normalization is 3 full-width DVE ops.
"""

import numpy as np
import ml_dtypes

BF16NP = ml_dtypes.bfloat16

B, S, D, H, DA, E = 2, 2048, 1024, 16, 64, 4
DFF = 4 * D
NCORES = 8
T = S // E  # 512 tokens per chunk / core
P = 128
SCALE = 1.0 / (np.sqrt(DA) * 12.0)
EPS = 1e-5
NDT = D // P      # 8 feature tiles
NQB = S // 512    # 4 query blocks per batch
NKT = S // P      # 16 key tiles per batch
NM1 = DFF // P    # 32 dff tiles
NTT = T // P      # 4 token tiles per chunk

_PROGRAM = None


def _build_program():
    from contextlib import ExitStack
    import concourse.bass as bass
    import concourse.mybir as mybir
    import concourse.tile as tile
    from concourse import bacc

    f32 = mybir.dt.float32
    bf = mybir.dt.bfloat16
    AF = mybir.ActivationFunctionType
    ALU = mybir.AluOpType

    nc = bacc.Bacc("TRN2", target_bir_lowering=False, debug=False,
                   num_devices=NCORES)

    def din(name, shape, dt):
        return nc.dram_tensor(name, shape, dt, kind="ExternalInput").ap()

    xT = din("xT", [B, NDT, P, S], bf)          # x transposed, both batches
    wq = din("wq", [NDT, P, P], bf)             # this core's 2 heads, [k,p,j]
    wk = din("wk", [NDT, P, P], bf)
    wv = din("wv", [NDT, P, P], bf)
    bqv = din("bq", [P, 1], f32)
    bkv = din("bk", [P, 1], f32)
    bvg = din("bvg", [P, 1], f32)               # gate * bv (2 heads)
    gate = din("gate", [P, 1], f32)             # residual gate, replicated
    tri = din("tri", [P, P], bf)                # tri[p,f] = f>=p
    iden = din("iden", [P, P], f32)
    onesc_f = din("onesc_f", [P, 1], f32)
    onesc_b = din("onesc_b", [P, 1], bf)
    onesr_f = din("onesr_f", [1, P], f32)
    xcT = din("xcT", [NDT, P, T], f32)          # residual x^T for my chunk
    lng = din("lng", [P, NDT], f32)
    lnb = din("lnb", [P, NDT], f32)
    w1 = din("w1", [NDT, P, DFF], bf)           # my expert W1 [k,p,m]
    b1v = din("b1", [P, NM1], f32)
    w2 = din("w2", [NM1, P, D], bf)             # my expert W2 [k,p,m]
    b2s = din("b2s", [P, NDT], f32)             # e_scalar * b2
    esv = din("es", [P, 1], f32)                # e_scalar replicated
    elng = din("elng", [P, NDT], f32)
    elnb = din("elnb", [P, NDT], f32)
    out_d = nc.dram_tensor("out", [NTT, P, D], f32, kind="ExternalOutput").ap()

    with tile.TileContext(nc) as tc, ExitStack() as ctx:
        cpool = ctx.enter_context(tc.tile_pool(name="const", bufs=1))
        xtp = ctx.enter_context(tc.tile_pool(name="xtp", bufs=NDT))

        # ---- attention-phase inputs first (DMA priority) ----
        xt_all = {}
        for b in range(B):
            for dt in range(NDT):
                t = xtp.tile([P, S], bf, tag="xt", name=f"xt{b}_{dt}")
                if b == 0:
                    nc.sync.dma_start(t[:], xT[b, dt])
                xt_all[(b, dt)] = t
        wq_sb = cpool.tile([P, NDT, P], bf)
        nc.sync.dma_start(wq_sb[:], wq.rearrange("k p j -> p k j"))
        wk_sb = cpool.tile([P, NDT, P], bf)
        nc.sync.dma_start(wk_sb[:], wk.rearrange("k p j -> p k j"))
        wv_sb = cpool.tile([P, NDT, P], bf)
        nc.sync.dma_start(wv_sb[:], wv.rearrange("k p j -> p k j"))
        bq_sb = cpool.tile([P, 1], f32)
        nc.sync.dma_start(bq_sb[:], bqv[:])
        bk_sb = cpool.tile([P, 1], f32)
        nc.sync.dma_start(bk_sb[:], bkv[:])
        bvg_sb = cpool.tile([P, 1], f32)
        nc.sync.dma_start(bvg_sb[:], bvg[:])
        gate_sb = cpool.tile([P, 1], f32)
        nc.sync.dma_start(gate_sb[:], gate[:])
        tri_sb = cpool.tile([P, P], bf)
        nc.sync.dma_start(tri_sb[:], tri[:])
        # batch-1 x loads queue behind the above
        for dt in range(NDT):
            nc.sync.dma_start(xt_all[(1, dt)][:], xT[1, dt])

        # ---- later-phase constants ----
        iden_sb = cpool.tile([P, P], f32)
        nc.sync.dma_start(iden_sb[:], iden[:])
        onescf_sb = cpool.tile([P, 1], f32)
        nc.sync.dma_start(onescf_sb[:], onesc_f[:])
        onescb_sb = cpool.tile([P, 1], bf)
        nc.sync.dma_start(onescb_sb[:], onesc_b[:])
        onesrf_sb = cpool.tile([1, P], f32)
        nc.sync.dma_start(onesrf_sb[:], onesr_f[:])
        lng_sb = cpool.tile([P, NDT], f32)
        nc.sync.dma_start(lng_sb[:], lng[:])
        lnb_sb = cpool.tile([P, NDT], f32)
        nc.sync.dma_start(lnb_sb[:], lnb[:])
        b1_sb = cpool.tile([P, NM1], f32)
        nc.sync.dma_start(b1_sb[:], b1v[:])
        b2s_sb = cpool.tile([P, NDT], f32)
        nc.sync.dma_start(b2s_sb[:], b2s[:])
        es_sb = cpool.tile([P, 1], f32)
        nc.sync.dma_start(es_sb[:], esv[:])
        elng_sb = cpool.tile([P, NDT], f32)
        nc.sync.dma_start(elng_sb[:], elng[:])
        elnb_sb = cpool.tile([P, NDT], f32)
        nc.sync.dma_start(elnb_sb[:], elnb[:])
        eps_sb = cpool.tile([1, 1], f32)
        nc.vector.memset(eps_sb[:], float(EPS))
        xc_sb = []
        for dt in range(NDT):
            t = cpool.tile([P, T], f32, tag="xc", bufs=NDT, name=f"xc{dt}")
            nc.sync.dma_start(t[:], xcT[dt])
            xc_sb.append(t)

        # a2a DRAM bounce buffers
        dpool = ctx.enter_context(
            tc.tile_pool(name="dramp", bufs=1, space="DRAM"))
        a_in = dpool.tile([NCORES, P, 512], bf)
        a_out = dpool.tile([NCORES, P, 512], bf)

        # =========== phase 1: projections + attention (per batch) ==========
        with tc.tile_pool(name="psA", bufs=1, space=bass.MemorySpace.PSUM) \
                as psA, \
             tc.tile_pool(name="qkp", bufs=2) as qkp, \
             tc.tile_pool(name="vp", bufs=NKT) as vp, \
             tc.tile_pool(name="ep", bufs=6) as epool, \
             tc.tile_pool(name="stgp", bufs=3) as stgp:
            for b in range(B):
                xt_b = [xt_all[(b, dt)] for dt in range(NDT)]

                # q^T, k^T: [128(2h x 64), S]
                qT = qkp.tile([P, S], bf, tag="qT", name=f"qT{b}")
                kT = qkp.tile([P, S], bf, tag="kT", name=f"kT{b}")
                for (w_sb, b_sb, oT) in ((wq_sb, bq_sb, qT),
                                         (wk_sb, bk_sb, kT)):
                    for qb in range(NQB):
                        q0 = 512 * qb
                        ps = psA.tile([P, 512], f32, tag="proj", bufs=2,
                                      name=f"pj{b}{qb}")
                        for k in range(NDT):
                            nc.tensor.matmul(
                                ps[:], w_sb[:, k, :], xt_b[k][:, q0:q0 + 512],
                                start=(k == 0), stop=(k == NDT - 1))
                        nc.vector.tensor_scalar_add(
                            oT[:, q0:q0 + 512], ps[:], b_sb[:])

                # v (token-major), 64 ones columns per head: [128, 2*128]
                # lhsT slice [v_h | ones] makes the AV matmul emit
                # [o^T_h ; rowsum x64] in one go.
                v_b = []
                for tt in range(NKT):
                    t0 = P * tt
                    ps = psA.tile([P, P], f32, tag="proj", bufs=2,
                                  name=f"pv{b}{tt}")
                    for k in range(NDT):
                        nc.tensor.matmul(
                            ps[:], xt_b[k][:, t0:t0 + P], wv_sb[:, k, :],
                            start=(k == 0), stop=(k == NDT - 1))
                    vt = vp.tile([P, 2 * P], bf, tag="v", name=f"v{b}_{tt}")
                    nc.vector.memset(vt[:], 1.0)
                    nc.vector.tensor_copy(vt[:, 0:64], ps[:, 0:64])
                    nc.vector.tensor_copy(vt[:, P:P + 64], ps[:, 64:128])
                    v_b.append(vt)

                # attention per qb with the two heads' chains interleaved;
                # destination core d = b*4 + qb
                for qb in range(NQB):
                    q0 = 512 * qb
                    nkt = 4 * (qb + 1)
                    stg = stgp.tile([P, 512], bf, tag="stg",
                                    name=f"stg{b}{qb}")
                    o_ps = [psA.tile([P, 512], f32, tag="o", bufs=2,
                                     name=f"o{b}{qb}{h}") for h in range(2)]
                    for kt in range(nkt):
                        k0 = P * kt
                        off = max(0, k0 - q0)
                        n = 512 - off
                        e_sbs = []
                        for h in range(2):
                            hp = h * 64
                            s_ps = psA.tile([P, 512], f32, tag="sc", bufs=4,
                                            name=f"s{b}{qb}{h}{kt}")
                            nc.tensor.matmul(
                                s_ps[:, 0:n],
                                kT[hp:hp + 64, k0:k0 + P],
                                qT[hp:hp + 64, q0 + off:q0 + 512],
                                start=True, stop=True)
                            e_sb = epool.tile([P, 512], bf, tag="exp",
                                              bufs=6, name=f"e{b}{qb}{h}{kt}")
                            nc.scalar.activation(
                                e_sb[:, 0:n], s_ps[:, 0:n], AF.Exp,
                                bias=0.0, scale=float(SCALE))
                            if k0 >= q0:  # diagonal block: causal mask
                                nc.vector.tensor_mul(
                                    e_sb[:, 0:P], e_sb[:, 0:P], tri_sb[:])
                            e_sbs.append(e_sb)
                        for h in range(2):
                            nc.tensor.matmul(
                                o_ps[h][:, off:512],
                                v_b[kt][:, h * P:(h + 1) * P],
                                e_sbs[h][:, 0:n],
                                start=(kt == 0), stop=(kt == nkt - 1))
                    for h in range(2):
                        hp = h * 64
                        # bounce rowsum to SBUF: the approx reciprocal's
                        # BITWISE_NOT seed needs raw IEEE fp32 bits, which
                        # the PSUM read path does not guarantee
                        rsum = epool.tile([64, 512], f32, tag="rsum",
                                          bufs=2, name=f"rw{b}{qb}{h}")
                        nc.vector.tensor_copy(rsum[:], o_ps[h][64:128, :])
                        recip = epool.tile([64, 512], f32, tag="recip",
                                           bufs=2, name=f"rc{b}{qb}{h}")
                        nc.vector.reciprocal_approx_fast(
                            recip[:], rsum[:])
                        # stage = (o * gate) * (1/rowsum) + gate*bv
                        nc.vector.scalar_tensor_tensor(
                            stg[hp:hp + 64, :], o_ps[h][0:64, :],
                            gate_sb[0:64, :], recip[:], ALU.mult, ALU.mult)
                        nc.vector.tensor_scalar_add(
                            stg[hp:hp + 64, :], stg[hp:hp + 64, :],
                            bvg_sb[hp:hp + 64, :])
                    nc.sync.dma_start(a_in[b * NQB + qb], stg[:])

        # =========== phase 2: AllToAll ==========
        nc.gpsimd.collective_compute(
            "AllToAll", mybir.AluOpType.bypass,
            replica_groups=[list(range(NCORES))],
            ins=[a_in[:].opt()], outs=[a_out[:].opt()])

        # =========== phase 3: residual + LN1 (feature-major) ==========
        x1f = []   # fp32, becomes x1 after LN
        x1b = []   # bf16 copy for FFN rhs
        lnp = ctx.enter_context(tc.tile_pool(name="lnp", bufs=1))
        aop = ctx.enter_context(tc.tile_pool(name="aop", bufs=4))
        smp2 = ctx.enter_context(tc.tile_pool(name="smp2", bufs=1))
        for dt in range(NDT):
            ao = aop.tile([P, 512], bf, tag="ao", name=f"ao{dt}")
            nc.sync.dma_start(ao[:], a_out[dt])
            xf = lnp.tile([P, T], f32, tag="x1f", bufs=NDT, name=f"x1f{dt}")
            nc.vector.tensor_add(xf[:], xc_sb[dt][:], ao[:])
            x1f.append(xf)
            x1b.append(lnp.tile([P, T], bf, tag="x1b", bufs=NDT,
                                name=f"x1b{dt}"))

        def ln_stats_mm(x_tiles, psum_pool, nm, dts):
            """Accumulate sum/sumsq over the given dt tiles (call once per
            dt group; first group allocates)."""
            for dt in dts:
                nc.tensor.matmul(ln_stats_mm.mean[nm][:], onescf_sb[:],
                                 x_tiles[dt][:],
                                 start=(dt == 0), stop=(dt == NDT - 1))
            for dt in dts:
                sq = smp2.tile([P, T], bf, tag="sqt", bufs=3,
                               name=f"sqt{nm}{dt}")
                nc.vector.tensor_mul(sq[:], x_tiles[dt][:], x_tiles[dt][:])
                nc.tensor.matmul(ln_stats_mm.sq[nm][:], onescb_sb[:], sq[:],
                                 start=(dt == 0), stop=(dt == NDT - 1))

        ln_stats_mm.mean = {}
        ln_stats_mm.sq = {}

        def ln_finish(psum_pool, nm):
            """Turn accumulated stats into replicated mu/rsig PSUM tiles."""
            mean_ps, sq_ps = ln_stats_mm.mean[nm], ln_stats_mm.sq[nm]
            mu = smp2.tile([1, 512], f32, tag="sm2", bufs=8, name=f"mu{nm}")
            nc.vector.tensor_scalar_mul(mu[:], mean_ps[:], 1.0 / D)
            ex2 = smp2.tile([1, 512], f32, tag="sm2", bufs=8, name=f"e2{nm}")
            nc.vector.tensor_scalar_mul(ex2[:], sq_ps[:], 1.0 / D)
            mu2 = smp2.tile([1, 512], f32, tag="sm2", bufs=8, name=f"m2{nm}")
            nc.vector.tensor_mul(mu2[:], mu[:], mu[:])
            var = smp2.tile([1, 512], f32, tag="sm2", bufs=8, name=f"vr{nm}")
            nc.vector.tensor_sub(var[:], ex2[:], mu2[:])
            sig = smp2.tile([1, 512], f32, tag="sm2", bufs=8, name=f"sg{nm}")
            nc.scalar.activation(sig[:], var[:], AF.Sqrt, bias=eps_sb[:])
            rsig = smp2.tile([1, 512], f32, tag="sm2", bufs=8,
                             name=f"rs{nm}")
            nc.vector.reciprocal_approx_fast(rsig[:], sig[:])
            mu_rep = psum_pool.tile([P, 512], f32, tag="rep2", bufs=2,
                                    name=f"mr{nm}")
            nc.tensor.matmul(mu_rep[:], onesrf_sb[:], mu[:],
                             start=True, stop=True)
            rs_rep = psum_pool.tile([P, 512], f32, tag="rep2", bufs=2,
                                    name=f"rr{nm}")
            nc.tensor.matmul(rs_rep[:], onesrf_sb[:], rsig[:],
                             start=True, stop=True)
            return mu_rep, rs_rep

        def ln_norm(x, mu_rep, rs_rep, g_sb, b_sb, dt, bf_out):
            nc.vector.tensor_sub(x[:], x[:], mu_rep[:])
            nc.vector.tensor_mul(x[:], x[:], rs_rep[:])
            nc.vector.tensor_scalar(
                x[:], x[:], g_sb[:, dt:dt + 1], b_sb[:, dt:dt + 1],
                ALU.mult, ALU.add)
            if bf_out is not None:
                nc.vector.tensor_copy(bf_out[:], x[:])

        with tc.tile_pool(name="psB", bufs=1,
                          space=bass.MemorySpace.PSUM) as psB:
            ln_stats_mm.mean["a"] = psB.tile([1, 512], f32, tag="red",
                                             bufs=2, name="mna")
            ln_stats_mm.sq["a"] = psB.tile([1, 512], f32, tag="red",
                                           bufs=2, name="sqa")
            ln_stats_mm(x1f, psB, "a", range(NDT))
            mu_rep, rs_rep = ln_finish(psB, "a")
            for dt in range(NDT):
                ln_norm(x1f[dt], mu_rep, rs_rep, lng_sb, lnb_sb, dt,
                        x1b[dt])

        # =========== phase 4: expert FFN ==========
        hp_pool = ctx.enter_context(tc.tile_pool(name="hT", bufs=NM1))
        hT = []
        zp = ctx.enter_context(tc.tile_pool(name="zp", bufs=NDT))
        wp = ctx.enter_context(tc.tile_pool(name="wp", bufs=1))
        # FFN1: groups of 4 dff-tiles; stream W1 slices
        with tc.tile_pool(name="psC", bufs=1,
                          space=bass.MemorySpace.PSUM) as psC:
            for mg in range(NM1 // 4):
                w1t = wp.tile([P, NDT, 512], bf, tag="w1", bufs=2,
                              name=f"w1t{mg}")
                nc.sync.dma_start(
                    w1t[:],
                    w1[:, :, mg * 512:(mg + 1) * 512]
                    .rearrange("k p j -> p k j"))
                fps = [psC.tile([P, T], f32, tag="f1", bufs=6,
                                name=f"f1_{mg}_{i}") for i in range(4)]
                for k in range(NDT):
                    for i in range(4):
                        nc.tensor.matmul(
                            fps[i][:], w1t[:, k, i * P:(i + 1) * P],
                            x1b[k][:],
                            start=(k == 0), stop=(k == NDT - 1))
                for i in range(4):
                    m = mg * 4 + i
                    ht = hp_pool.tile([P, T], bf, tag="hT", name=f"hT{m}")
                    nc.scalar.activation(ht[:], fps[i][:], AF.Gelu,
                                         bias=b1_sb[:, m:m + 1], scale=1.0)
                    hT.append(ht)

        # FFN2 in two 4-tile halves so LN2 stats overlap the second half
        z = [None] * NDT
        with tc.tile_pool(name="psE", bufs=1,
                          space=bass.MemorySpace.PSUM) as psE:
            ln_stats_mm.mean["b"] = psE.tile([1, 512], f32, tag="red",
                                             bufs=2, name="mnb")
            ln_stats_mm.sq["b"] = psE.tile([1, 512], f32, tag="red",
                                           bufs=2, name="sqb")
            with tc.tile_pool(name="psD", bufs=1,
                              space=bass.MemorySpace.PSUM) as psD:
                for half in range(2):
                    dts = [half * 4 + i for i in range(4)]
                    yps = [psD.tile([P, T], f32, tag="f2", bufs=4,
                                    name=f"y{dt}") for dt in dts]
                    for k in range(NM1):
                        w2t = wp.tile([P, D], bf, tag="w2", bufs=3,
                                      name=f"w2t{half}_{k}")
                        nc.sync.dma_start(w2t[:], w2[k])
                        for i, dt in enumerate(dts):
                            nc.tensor.matmul(
                                yps[i][:], w2t[:, dt * P:(dt + 1) * P],
                                hT[k][:],
                                start=(k == 0), stop=(k == NM1 - 1))
                    for i, dt in enumerate(dts):
                        zt = zp.tile([P, T], f32, tag="z", bufs=NDT,
                                     name=f"z{dt}")
                        # z = es*y + x1 (+ es*b2)
                        nc.vector.scalar_tensor_tensor(
                            zt[:], yps[i][:], es_sb[:], x1f[dt][:],
                            ALU.mult, ALU.add)
                        nc.vector.tensor_scalar_add(
                            zt[:], zt[:], b2s_sb[:, dt:dt + 1])
                        z[dt] = zt
                    # LN2 stats for this half overlap the next half's MMs
                    ln_stats_mm(z, psE, "b", dts)

            # =========== phase 5: LN2 + transpose + output ==========
            mu2r, rs2r = ln_finish(psE, "b")
            with tc.tile_pool(name="psF", bufs=1,
                              space=bass.MemorySpace.PSUM) as psF, \
                 tc.tile_pool(name="outp", bufs=NTT) as outp:
                ot = [outp.tile([P, D], f32, tag="ot", name=f"ot{tt}")
                      for tt in range(NTT)]
                for dt in range(NDT):
                    ln_norm(z[dt], mu2r, rs2r, elng_sb, elnb_sb, dt, None)
                    for tt in range(NTT):
                        tp = psF.tile([P, P], f32, tag="tr", bufs=4,
                                      name=f"tr{tt}{dt}")
                        nc.tensor.transpose(
                            tp[:], z[dt][:, tt * P:(tt + 1) * P], iden_sb[:])
                        nc.vector.tensor_copy(
                            ot[tt][:, dt * P:(dt + 1) * P], tp[:])
                for tt in range(NTT):
                    nc.sync.dma_start(out_d[tt], ot[tt][:])

    nc.compile()
    return nc


def _get_program():
    global _PROGRAM
    if _PROGRAM is None:
        _PROGRAM = _build_program()
    return _PROGRAM


def _host_prep(inputs):
    """Shard + lay out inputs for each of the 8 cores."""
    x = np.asarray(inputs["x"], np.float32)
    Wq = np.asarray(inputs["Wq"], np.float32)
    bq = np.asarray(inputs["bq"], np.float32)
    Wk = np.asarray(inputs["Wk"], np.float32)
    bk = np.asarray(inputs["bk"], np.float32)
    Wv = np.asarray(inputs["Wv"], np.float32)
    bv = np.asarray(inputs["bv"], np.float32)
    scalar = np.float32(inputs["scalar"])
    ln_g = np.asarray(inputs["ln_g"], np.float32)
    ln_b = np.asarray(inputs["ln_b"], np.float32)
    eW1 = np.asarray(inputs["eW1"], np.float32)
    eb1 = np.asarray(inputs["eb1"], np.float32)
    eW2 = np.asarray(inputs["eW2"], np.float32)
    eb2 = np.asarray(inputs["eb2"], np.float32)
    e_scalar = np.asarray(inputs["e_scalar"], np.float32)
    eln_g = np.asarray(inputs["eln_g"], np.float32)
    eln_b = np.asarray(inputs["eln_b"], np.float32)

    xT_all = np.ascontiguousarray(x.transpose(0, 2, 1)).reshape(B, NDT, P, S)
    xT_bf = xT_all.astype(BF16NP)
    tri = (np.arange(P)[None, :] >= np.arange(P)[:, None])
    iden = np.eye(P, dtype=np.float32)

    def col(v):
        return np.ascontiguousarray(v.reshape(-1, 1), dtype=np.float32)

    def pk(v):  # [D]-like -> [P, n]
        n = v.size // P
        return np.ascontiguousarray(v.reshape(n, P).T, dtype=np.float32)

    in_maps = []
    for c in range(NCORES):
        h0 = 2 * c
        b_out, e_out = c // NQB, c % NQB
        t0 = e_out * T
        wq_c = np.concatenate([Wq[h0], Wq[h0 + 1]], axis=1)  # [1024,128]
        wk_c = np.concatenate([Wk[h0], Wk[h0 + 1]], axis=1)
        wv_c = np.concatenate([Wv[h0], Wv[h0 + 1]], axis=1)
        bq_c = np.concatenate([bq[h0], bq[h0 + 1]])
        bk_c = np.concatenate([bk[h0], bk[h0 + 1]])
        bv_c = np.concatenate([bv[h0], bv[h0 + 1]])
        xc = np.ascontiguousarray(x[b_out, t0:t0 + T, :].T)  # [1024, 512]
        m = {
            "xT": xT_bf,
            "wq": np.ascontiguousarray(wq_c.reshape(NDT, P, P), BF16NP),
            "wk": np.ascontiguousarray(wk_c.reshape(NDT, P, P), BF16NP),
            "wv": np.ascontiguousarray(wv_c.reshape(NDT, P, P), BF16NP),
            "bq": col(bq_c),
            "bk": col(bk_c),
            "bvg": col(scalar * bv_c),
            "gate": np.full((P, 1), scalar, np.float32),
            "tri": tri.astype(BF16NP),
            "iden": iden,
            "onesc_f": np.ones((P, 1), np.float32),
            "onesc_b": np.ones((P, 1), BF16NP),
            "onesr_f": np.ones((1, P), np.float32),
            "xcT": np.ascontiguousarray(xc.reshape(NDT, P, T), np.float32),
            "lng": pk(ln_g),
            "lnb": pk(ln_b),
            "w1": np.ascontiguousarray(
                eW1[e_out].reshape(NDT, P, DFF), BF16NP),
            "b1": pk(eb1[e_out]),
            "w2": np.ascontiguousarray(
                eW2[e_out].reshape(NM1, P, D), BF16NP),
            "b2s": pk(e_scalar[e_out] * eb2[e_out]),
            "es": np.full((P, 1), e_scalar[e_out], np.float32),
            "elng": pk(eln_g[e_out]),
            "elnb": pk(eln_b[e_out]),
        }
        in_maps.append(m)
    return in_maps


_LAST_RESULT = {}


def kernel(**inputs) -> np.ndarray:
    import os
    from concourse.bass_utils import run_bass_kernel_spmd

    nc = _get_program()
    in_maps = _host_prep(inputs)
    trace = bool(int(os.environ.get("KBENCH_TRACE", "0")))
    res = run_bass_kernel_spmd(nc, in_maps, core_ids=list(range(NCORES)),
                               trace=trace)
    _LAST_RESULT["exec_time_ns"] = res.exec_time_ns
    _LAST_RESULT["res"] = res

    out = np.empty((B, S, D), np.float32)
    for c in range(NCORES):
        b_out, e_out = c // NQB, c % NQB
        chunk = np.asarray(res.results[c]["out"], np.float32)
        out[b_out, e_out * T:(e_out + 1) * T, :] = chunk.reshape(T, D)
    return out
